# revision 45
# baseline (speedup 1.0000x reference)
"""Bahdanau-attention kernel for Trainium2 (8 NeuronCores, Bass/Tile).

Computation (reference, fp32):
    Wh  = hidden @ W_w.T + W_b                      # [B, H]
    Ue  = einsum('bse,he->bsh', enc^T, U_w) + U_b   # [B, S, H]
    en  = tanh(Wh[:,None,:] + Ue) @ v_w[0]          # [B, S]
    out = softmax(where(mask, -1e10, en), axis=1)

Strategy
- Data-parallel over batch: 8 batches per core, weights replicated.
- Masked positions contribute exactly 0 to the softmax (exp(-1e10) = 0
  in fp32), so the host packs only the unmasked s-columns per batch row
  (padded to NP = max unmasked count rounded to a multiple of 4) and
  scatters results back; the device computes energies only for packed
  columns. This is exact, not an approximation. Fully-masked rows are
  uniform 1/S by definition and fixed up on the host.
- Main matmul out[h, s] = U_w.T-chunk (stationary) x enc-chunk (moving)
  in bf16 with fp32 PSUM accumulation; 16 k-chunks of 128 accumulate in
  one PSUM bank per (batch, h-chunk). Weights are host-rechunked per
  h-chunk so the first main block only needs 0.5 MB of weight DMA.
- Wh + W_b + U_b is folded into the tanh as a per-partition ACT bias;
  the Wh chains interleave with batch 0's main blocks so PE work paces
  the startup DMA stream (which is HBM-bandwidth-bound).
- The v-dots are M=1 matmuls accumulated over h-chunks, batched at the
  end of each batch so the main matmuls stay dense (no per-chunk weight
  switching).
- Per-row softmax runs on partition 0 (no max-subtraction needed:
  |energy| < 32 so fp32 exp is safe; masked/padded columns give 0).

Host-side prep only reshapes/retypes/packs inputs; all FLOPs of the
module run on device in bf16/fp32.
"""

import numpy as np
import ml_dtypes

B, S, H, E = 64, 512, 1024, 2048
NCORES = 8
BL = B // NCORES          # batches per core
HC = H // 128             # h chunks
EC = E // 128             # e (contraction) chunks
KC = H // 128             # k chunks for the Wh matmul
NEG = np.float32(-1e10)

bf16 = ml_dtypes.bfloat16

_CACHE = {}


def _build_nc(NP):
    """Per-core program; NP = packed s-width (padded s-width, multiple of 4, <= 512)."""
    import concourse.mybir as mybir
    import concourse.tile as tile
    from concourse import bacc

    F32 = mybir.dt.float32
    BF = mybir.dt.bfloat16
    AF = mybir.ActivationFunctionType

    nc = bacc.Bacc(num_swdge_queues=4)
    enc_t = nc.declare_dram_parameter("enc_t", [E, BL, NP], BF, isOutput=False)
    # U_w.T pre-chunked by h-chunk: [hc, p(=e%128), ec, v(=h%128)], so the
    # first main block only needs the hc=0 slice (0.5 MB) instead of 4 MB
    uwT = nc.declare_dram_parameter("uwT", [HC, 128, EC, 128], BF, isOutput=False)
    # W_w.T re-chunked the same way: [hc, p(=k%128), kc, v(=h%128)]
    wwT = nc.declare_dram_parameter("wwT", [HC, 128, KC, 128], BF, isOutput=False)
    hidT = nc.declare_dram_parameter("hidT", [128, KC * BL], BF, isOutput=False)
    vt = nc.declare_dram_parameter("vt", [128, HC], BF, isOutput=False)
    bc = nc.declare_dram_parameter("bc", [128, HC], F32, isOutput=False)
    amask = nc.declare_dram_parameter("amask", [1, BL * NP], F32, isOutput=False)
    out_d = nc.declare_dram_parameter("out", [1, BL * NP], F32, isOutput=True)

    enc_r = enc_t.rearrange("(ec p) b s -> ec p b s", p=128)

    ENC_BUFS = 44

    with tile.TileContext(nc) as tc:
        with (
            tc.tile_pool(name="const", bufs=1) as cst,
            tc.tile_pool(name="wpool", bufs=1) as wp,
            tc.tile_pool(name="encp", bufs=ENC_BUFS) as encp,
            tc.tile_pool(name="thp", bufs=10) as thp,
            tc.tile_pool(name="smp", bufs=4) as smp,
            tc.tile_pool(name="pup", bufs=5, space="PSUM") as pup,
            tc.tile_pool(name="pep", bufs=2, space="PSUM") as pep,
            tc.tile_pool(name="pwp", bufs=1, space="PSUM") as pwp,
        ):
            # ---- constants / weights -------------------------------------
            # DMA order matters for the startup critical path: the Wh
            # prologue needs hid+ww first; the first main block needs uw.
            hid_sb = cst.tile([128, KC * BL], BF, tag="hid")
            nc.sync.dma_start(hid_sb[:], hidT[:])

            # HAM warmup: the PE clock gate defaults to half rate until it
            # has seen ~3.4us of sustained activity. Dummy matmuls on memset
            # data (no DMA dependency) use the otherwise idle startup window
            # so the first real matmuls run at full clock.
            warm_sb = cst.tile([128, 272], BF, tag="warm")
            nc.vector.memset(warm_sb[:], 0.0)
            pwarm = pwp.tile([128, 144], F32, tag="pw")
            for _ in range(40):
                nc.tensor.matmul(
                    pwarm[:], lhsT=warm_sb[:, 0:128], rhs=warm_sb[:, 128:272],
                    start=True, stop=True,
                )

            # per-hc weight chunks: each Wh chain / main block only needs its
            # own chunk, so PE work starts after a few hundred KB of DMA and
            # the rest streams in behind it. DMA order matches the b=0
            # interleave: (ww0, uw0, enc0) first, then (ww_k, uw_k) pairs.
            ww_sb = []
            for hc in range(HC):
                t = wp.tile([128, KC * 128], BF, tag=f"ww{hc}")
                ww_sb.append(t)
            uw_sb = []
            for hc in range(HC):
                t = wp.tile([128, EC * 128], BF, tag=f"uw{hc}")
                uw_sb.append(t)
            HALF = EC * 128 // 2
            KHALF = KC * 128 // 2

            nc.sync.dma_start(ww_sb[0][:, 0:KHALF], wwT[0, :, 0:KC // 2, :])
            nc.gpsimd.dma_start(ww_sb[0][:, KHALF:], wwT[0, :, KC // 2:, :])
            nc.sync.dma_start(uw_sb[0][:, 0:HALF], uwT[0, :, 0:EC // 2, :])
            nc.gpsimd.dma_start(uw_sb[0][:, HALF:], uwT[0, :, EC // 2:, :])
            bc_sb = cst.tile([128, HC], F32, tag="bc")
            nc.gpsimd.dma_start(bc_sb[:], bc[:])

            enc0_tiles = []
            for ec in range(EC):
                t = encp.tile([128, NP], BF, tag="enc")
                eng = (nc.sync, nc.gpsimd)[ec % 2]
                eng.dma_start(t[:], enc_r[ec, :, 0, :])
                enc0_tiles.append(t)

            for hc in range(1, HC):
                eng = (nc.sync, nc.gpsimd)[hc % 2]
                eng2 = (nc.gpsimd, nc.sync)[hc % 2]
                eng.dma_start(ww_sb[hc][:], wwT[hc])
                eng.dma_start(uw_sb[hc][:, 0:HALF], uwT[hc, :, 0:EC // 2, :])
                eng2.dma_start(uw_sb[hc][:, HALF:], uwT[hc, :, EC // 2:, :])
            vt_sb = cst.tile([128, HC], BF, tag="vt")
            nc.gpsimd.dma_start(vt_sb[:], vt[:])
            am_sb = cst.tile([1, BL * NP], F32, tag="am")
            nc.gpsimd.dma_start(am_sb[:], amask[:])

            bias_sb = cst.tile([128, HC * BL], F32, tag="bias")
            en_sb = cst.tile([1, BL * NP], F32, tag="en")
            res_sb = cst.tile([1, BL * NP], F32, tag="res")

            # ---- main loop over local batches ----------------------------
            # b=0 interleaves the Wh/bias prologue chain-by-chain with its
            # own main blocks so PE work paces with the weight DMA stream.
            for b in range(BL):
                if b == 0:
                    enc_tiles = enc0_tiles
                else:
                    enc_tiles = []
                    for ec in range(EC):
                        t = encp.tile([128, NP], BF, tag="enc")
                        eng = nc.sync if ec % 2 == 0 else nc.gpsimd
                        eng.dma_start(t[:], enc_r[ec, :, b, :])
                        enc_tiles.append(t)

                pe_ = pep.tile([1, NP], F32, tag="pe")
                th_tiles = []
                for hc in range(HC):
                    if b == 0:
                        # Wh chain for this h-chunk, feeding the tanh bias
                        pw = pwp.tile([128, BL], F32, tag="pw")
                        for kc in range(KC):
                            nc.tensor.matmul(
                                pw[:],
                                lhsT=ww_sb[hc][:, kc * 128:(kc + 1) * 128],
                                rhs=hid_sb[:, kc * BL:(kc + 1) * BL],
                                start=(kc == 0),
                                stop=(kc == KC - 1),
                            )
                        nc.vector.tensor_tensor(
                            bias_sb[:, hc * BL:(hc + 1) * BL], pw[:],
                            bc_sb[:, hc:hc + 1].to_broadcast([128, BL]),
                            mybir.AluOpType.add,
                        )
                    pu = pup.tile([128, NP], F32, tag="pu")
                    for ec in range(EC):
                        nc.tensor.matmul(
                            pu[:],
                            lhsT=uw_sb[hc][:, ec * 128:(ec + 1) * 128],
                            rhs=enc_tiles[ec][:],
                            start=(ec == 0),
                            stop=(ec == EC - 1),
                        )
                    th = thp.tile([128, NP], BF, tag="th")
                    nc.scalar.activation(
                        th[:], pu[:], AF.Tanh,
                        bias=bias_sb[:, hc * BL + b:hc * BL + b + 1],
                    )
                    th_tiles.append(th)
                # batched v-dots: one weight-switch region per batch instead
                # of one per h-chunk keeps the main matmuls dense
                for hc in range(HC):
                    nc.tensor.matmul(
                        pe_[0:1, :],
                        lhsT=vt_sb[:, hc:hc + 1],
                        rhs=th_tiles[hc][:],
                        start=(hc == 0),
                        stop=(hc == HC - 1),
                    )

                # ---- mask + softmax over packed columns on partition 0 ---
                sl = slice(b * NP, (b + 1) * NP)
                nc.vector.tensor_add(en_sb[0:1, sl], pe_[0:1, :], am_sb[0:1, sl])
                ssum = smp.tile([1, 1], F32, tag="ssum")
                nc.scalar.activation(
                    res_sb[0:1, sl], en_sb[0:1, sl], AF.Exp,
                    accum_out=ssum[0:1, 0:1],
                )
                rcp = smp.tile([1, 1], F32, tag="rcp")
                nc.vector.reciprocal(rcp[0:1, :], ssum[0:1, :])
                nc.vector.tensor_tensor(
                    res_sb[0:1, sl], res_sb[0:1, sl],
                    rcp[0:1, 0:1].to_broadcast([1, NP]),
                    mybir.AluOpType.mult,
                )
                nc.sync.dma_start(out_d[0:1, sl], res_sb[0:1, sl])

    nc.finalize()
    return nc


def _prep_inputs(hidden, encoder_outputs, mask, W_w, W_b, U_w, U_b, v_w):
    enc_bf = encoder_outputs.astype(bf16)          # [S, B, E]
    uwT_np = np.ascontiguousarray(U_w.T).astype(bf16)
    # re-chunk U_w.T [E, H] -> [hc, p, ec, v]: (e=ec*128+p, h=hc*128+v)
    uwT_np = np.ascontiguousarray(
        uwT_np.reshape(EC, 128, HC, 128).transpose(2, 1, 0, 3))
    wwT_np = np.ascontiguousarray(W_w.T).astype(bf16)
    wwT_np = np.ascontiguousarray(
        wwT_np.reshape(KC, 128, HC, 128).transpose(2, 1, 0, 3))
    vt_np = np.ascontiguousarray(v_w[0].reshape(HC, 128).T).astype(bf16)
    bc_np = np.ascontiguousarray((W_b + U_b).reshape(HC, 128).T).astype(np.float32)

    idx_all = [np.nonzero(~mask[i])[0] for i in range(B)]
    counts = np.array([len(ix) for ix in idx_all])
    NP = int(max(64, 4 * -(-counts.max() // 4)))  # ceil to multiple of 4

    in_maps = []
    for c in range(NCORES):
        bsl = slice(c * BL, (c + 1) * BL)
        enc_c = np.ascontiguousarray(enc_bf[:, bsl, :].transpose(2, 1, 0))  # [E, BL, S]
        enc_p = np.zeros((E, BL, NP), bf16)
        am_p = np.full((BL, NP), NEG, np.float32)
        for b in range(BL):
            ix = idx_all[c * BL + b]
            cnt = len(ix)
            if cnt:
                enc_p[:, b, :cnt] = enc_c[:, b, ix]
                am_p[b, :cnt] = 0.0
        hid_c = hidden[bsl].astype(bf16)                                    # [BL, H]
        hidT_c = np.ascontiguousarray(
            hid_c.T.reshape(KC, 128, BL).transpose(1, 0, 2)
        ).reshape(128, KC * BL)
        in_maps.append({
            "enc_t": enc_p,
            "uwT": uwT_np,
            "wwT": wwT_np,
            "hidT": hidT_c,
            "vt": vt_np,
            "bc": bc_np,
            "amask": am_p.reshape(1, BL * NP),
        })
    return in_maps, NP, idx_all, counts


def _run(in_maps, NP, trace=False):
    from concourse import bass_utils
    if NP not in _CACHE:
        _CACHE[NP] = _build_nc(NP)
    nc = _CACHE[NP]
    return bass_utils.run_bass_kernel_spmd(
        nc, in_maps, core_ids=list(range(NCORES)), trace=trace
    )


def kernel(hidden, encoder_outputs, mask, W_w, W_b, U_w, U_b, v_w,
           _trace=False, _return_bkr=False):
    hidden = np.asarray(hidden, dtype=np.float32)
    encoder_outputs = np.asarray(encoder_outputs, dtype=np.float32)
    mask = np.asarray(mask).astype(bool)
    W_w = np.asarray(W_w, dtype=np.float32)
    W_b = np.asarray(W_b, dtype=np.float32)
    U_w = np.asarray(U_w, dtype=np.float32)
    U_b = np.asarray(U_b, dtype=np.float32)
    v_w = np.asarray(v_w, dtype=np.float32)

    in_maps, NP, idx_all, counts = _prep_inputs(
        hidden, encoder_outputs, mask, W_w, W_b, U_w, U_b, v_w)
    bkr = _run(in_maps, NP, trace=_trace)

    out = np.zeros((B, S), np.float32)
    for c in range(NCORES):
        dev = bkr.results[c]["out"].reshape(BL, NP)
        for b in range(BL):
            i = c * BL + b
            cnt = counts[i]
            if cnt:
                out[i, idx_all[i]] = dev[b, :cnt]
            else:
                # fully-masked row: softmax over all -1e10 is uniform
                out[i, :] = np.float32(1.0 / S)
    if _return_bkr:
        return out, bkr
    return out


# revision 46
# speedup vs baseline: 1.0063x; 1.0063x over previous
"""Bahdanau-attention kernel for Trainium2 (8 NeuronCores, Bass/Tile).

Computation (reference, fp32):
    Wh  = hidden @ W_w.T + W_b                      # [B, H]
    Ue  = einsum('bse,he->bsh', enc^T, U_w) + U_b   # [B, S, H]
    en  = tanh(Wh[:,None,:] + Ue) @ v_w[0]          # [B, S]
    out = softmax(where(mask, -1e10, en), axis=1)

Strategy
- Data-parallel over batch: 8 batches per core, weights replicated.
- Masked positions contribute exactly 0 to the softmax (exp(-1e10) = 0
  in fp32), so the host packs only the unmasked s-columns per batch row
  (padded to NP = max unmasked count rounded to a multiple of 4) and
  scatters results back; the device computes energies only for packed
  columns. This is exact, not an approximation. Fully-masked rows are
  uniform 1/S by definition and fixed up on the host.
- Main matmul out[h, s] = U_w.T-chunk (stationary) x enc-chunk (moving)
  in bf16 with fp32 PSUM accumulation; 16 k-chunks of 128 accumulate in
  one PSUM bank per (batch, h-chunk). Weights are host-rechunked per
  h-chunk so the first main block only needs 0.5 MB of weight DMA.
- Wh + W_b + U_b is folded into the tanh as a per-partition ACT bias;
  the Wh chains interleave with batch 0's main blocks so PE work paces
  the startup DMA stream (which is HBM-bandwidth-bound).
- The v-dots are M=1 matmuls accumulated over h-chunks, batched at the
  end of each batch so the main matmuls stay dense (no per-chunk weight
  switching).
- Per-row softmax runs on partition 0 (no max-subtraction needed:
  |energy| < 32 so fp32 exp is safe; masked/padded columns give 0).

Host-side prep only reshapes/retypes/packs inputs; all FLOPs of the
module run on device in bf16/fp32.
"""

import numpy as np
import ml_dtypes

B, S, H, E = 64, 512, 1024, 2048
NCORES = 8
BL = B // NCORES          # batches per core
HC = H // 128             # h chunks
EC = E // 128             # e (contraction) chunks
KC = H // 128             # k chunks for the Wh matmul
NEG = np.float32(-1e10)

bf16 = ml_dtypes.bfloat16

_CACHE = {}


def _build_nc(NP):
    """Per-core program; NP = packed s-width (padded s-width, multiple of 4, <= 512)."""
    import concourse.mybir as mybir
    import concourse.tile as tile
    from concourse import bacc

    F32 = mybir.dt.float32
    BF = mybir.dt.bfloat16
    AF = mybir.ActivationFunctionType

    nc = bacc.Bacc(num_swdge_queues=4)
    enc_t = nc.declare_dram_parameter("enc_t", [E, BL, NP], BF, isOutput=False)
    # U_w.T pre-chunked by h-chunk: [hc, p(=e%128), ec, v(=h%128)], so the
    # first main block only needs the hc=0 slice (0.5 MB) instead of 4 MB
    uwT = nc.declare_dram_parameter("uwT", [HC, 128, EC, 128], BF, isOutput=False)
    # W_w.T re-chunked the same way: [hc, p(=k%128), kc, v(=h%128)]
    wwT = nc.declare_dram_parameter("wwT", [HC, 128, KC, 128], BF, isOutput=False)
    hidT = nc.declare_dram_parameter("hidT", [128, KC * BL], BF, isOutput=False)
    vt = nc.declare_dram_parameter("vt", [128, HC], BF, isOutput=False)
    bc = nc.declare_dram_parameter("bc", [128, HC], F32, isOutput=False)
    amask = nc.declare_dram_parameter("amask", [1, BL * NP], F32, isOutput=False)
    out_d = nc.declare_dram_parameter("out", [1, BL * NP], F32, isOutput=True)

    enc_r = enc_t.rearrange("(ec p) b s -> ec p b s", p=128)

    ENC_BUFS = 44

    with tile.TileContext(nc) as tc:
        with (
            tc.tile_pool(name="const", bufs=1) as cst,
            tc.tile_pool(name="wpool", bufs=1) as wp,
            tc.tile_pool(name="encp", bufs=ENC_BUFS) as encp,
            tc.tile_pool(name="thp", bufs=10) as thp,
            tc.tile_pool(name="smp", bufs=4) as smp,
            tc.tile_pool(name="pup", bufs=5, space="PSUM") as pup,
            tc.tile_pool(name="pep", bufs=2, space="PSUM") as pep,
            tc.tile_pool(name="pwp", bufs=1, space="PSUM") as pwp,
        ):
            # ---- constants / weights -------------------------------------
            # DMA order matters for the startup critical path: the Wh
            # prologue needs hid+ww first; the first main block needs uw.
            hid_sb = cst.tile([128, KC * BL], BF, tag="hid")
            nc.sync.dma_start(hid_sb[:], hidT[:])

            # per-hc weight chunks: each Wh chain / main block only needs its
            # own chunk, so PE work starts after a few hundred KB of DMA and
            # the rest streams in behind it. DMA order matches the b=0
            # interleave: (ww0, uw0, enc0) first, then (ww_k, uw_k) pairs.
            ww_sb = []
            for hc in range(HC):
                t = wp.tile([128, KC * 128], BF, tag=f"ww{hc}")
                ww_sb.append(t)
            uw_sb = []
            for hc in range(HC):
                t = wp.tile([128, EC * 128], BF, tag=f"uw{hc}")
                uw_sb.append(t)
            HALF = EC * 128 // 2
            KHALF = KC * 128 // 2

            nc.sync.dma_start(ww_sb[0][:, 0:KHALF], wwT[0, :, 0:KC // 2, :])
            nc.gpsimd.dma_start(ww_sb[0][:, KHALF:], wwT[0, :, KC // 2:, :])
            nc.sync.dma_start(uw_sb[0][:, 0:HALF], uwT[0, :, 0:EC // 2, :])
            nc.gpsimd.dma_start(uw_sb[0][:, HALF:], uwT[0, :, EC // 2:, :])
            bc_sb = cst.tile([128, HC], F32, tag="bc")
            nc.gpsimd.dma_start(bc_sb[:], bc[:])

            enc0_tiles = []
            for ec in range(EC):
                t = encp.tile([128, NP], BF, tag="enc")
                eng = (nc.sync, nc.gpsimd)[ec % 2]
                eng.dma_start(t[:], enc_r[ec, :, 0, :])
                enc0_tiles.append(t)

            for hc in range(1, HC):
                eng = (nc.sync, nc.gpsimd)[hc % 2]
                eng2 = (nc.gpsimd, nc.sync)[hc % 2]
                eng.dma_start(ww_sb[hc][:], wwT[hc])
                eng.dma_start(uw_sb[hc][:, 0:HALF], uwT[hc, :, 0:EC // 2, :])
                eng2.dma_start(uw_sb[hc][:, HALF:], uwT[hc, :, EC // 2:, :])
            vt_sb = cst.tile([128, HC], BF, tag="vt")
            nc.gpsimd.dma_start(vt_sb[:], vt[:])
            am_sb = cst.tile([1, BL * NP], F32, tag="am")
            nc.gpsimd.dma_start(am_sb[:], amask[:])

            bias_sb = cst.tile([128, HC * BL], F32, tag="bias")
            en_sb = cst.tile([1, BL * NP], F32, tag="en")
            res_sb = cst.tile([1, BL * NP], F32, tag="res")

            # ---- main loop over local batches ----------------------------
            # b=0 interleaves the Wh/bias prologue chain-by-chain with its
            # own main blocks so PE work paces with the weight DMA stream.
            for b in range(BL):
                if b == 0:
                    enc_tiles = enc0_tiles
                else:
                    enc_tiles = []
                    for ec in range(EC):
                        t = encp.tile([128, NP], BF, tag="enc")
                        eng = nc.sync if ec % 2 == 0 else nc.gpsimd
                        eng.dma_start(t[:], enc_r[ec, :, b, :])
                        enc_tiles.append(t)

                pe_ = pep.tile([1, NP], F32, tag="pe")
                th_tiles = []
                for hc in range(HC):
                    if b == 0:
                        # Wh chain for this h-chunk, feeding the tanh bias
                        pw = pwp.tile([128, BL], F32, tag="pw")
                        for kc in range(KC):
                            nc.tensor.matmul(
                                pw[:],
                                lhsT=ww_sb[hc][:, kc * 128:(kc + 1) * 128],
                                rhs=hid_sb[:, kc * BL:(kc + 1) * BL],
                                start=(kc == 0),
                                stop=(kc == KC - 1),
                            )
                        nc.vector.tensor_tensor(
                            bias_sb[:, hc * BL:(hc + 1) * BL], pw[:],
                            bc_sb[:, hc:hc + 1].to_broadcast([128, BL]),
                            mybir.AluOpType.add,
                        )
                    pu = pup.tile([128, NP], F32, tag="pu")
                    for ec in range(EC):
                        nc.tensor.matmul(
                            pu[:],
                            lhsT=uw_sb[hc][:, ec * 128:(ec + 1) * 128],
                            rhs=enc_tiles[ec][:],
                            start=(ec == 0),
                            stop=(ec == EC - 1),
                        )
                    th = thp.tile([128, NP], BF, tag="th")
                    nc.scalar.activation(
                        th[:], pu[:], AF.Tanh,
                        bias=bias_sb[:, hc * BL + b:hc * BL + b + 1],
                    )
                    th_tiles.append(th)
                # batched v-dots: one weight-switch region per batch instead
                # of one per h-chunk keeps the main matmuls dense
                for hc in range(HC):
                    nc.tensor.matmul(
                        pe_[0:1, :],
                        lhsT=vt_sb[:, hc:hc + 1],
                        rhs=th_tiles[hc][:],
                        start=(hc == 0),
                        stop=(hc == HC - 1),
                    )

                # ---- mask + softmax over packed columns on partition 0 ---
                sl = slice(b * NP, (b + 1) * NP)
                nc.vector.tensor_add(en_sb[0:1, sl], pe_[0:1, :], am_sb[0:1, sl])
                ssum = smp.tile([1, 1], F32, tag="ssum")
                nc.scalar.activation(
                    res_sb[0:1, sl], en_sb[0:1, sl], AF.Exp,
                    accum_out=ssum[0:1, 0:1],
                )
                rcp = smp.tile([1, 1], F32, tag="rcp")
                nc.vector.reciprocal(rcp[0:1, :], ssum[0:1, :])
                nc.vector.tensor_tensor(
                    res_sb[0:1, sl], res_sb[0:1, sl],
                    rcp[0:1, 0:1].to_broadcast([1, NP]),
                    mybir.AluOpType.mult,
                )
                nc.sync.dma_start(out_d[0:1, sl], res_sb[0:1, sl])

    nc.finalize()
    return nc


def _prep_inputs(hidden, encoder_outputs, mask, W_w, W_b, U_w, U_b, v_w):
    enc_bf = encoder_outputs.astype(bf16)          # [S, B, E]
    uwT_np = np.ascontiguousarray(U_w.T).astype(bf16)
    # re-chunk U_w.T [E, H] -> [hc, p, ec, v]: (e=ec*128+p, h=hc*128+v)
    uwT_np = np.ascontiguousarray(
        uwT_np.reshape(EC, 128, HC, 128).transpose(2, 1, 0, 3))
    wwT_np = np.ascontiguousarray(W_w.T).astype(bf16)
    wwT_np = np.ascontiguousarray(
        wwT_np.reshape(KC, 128, HC, 128).transpose(2, 1, 0, 3))
    vt_np = np.ascontiguousarray(v_w[0].reshape(HC, 128).T).astype(bf16)
    bc_np = np.ascontiguousarray((W_b + U_b).reshape(HC, 128).T).astype(np.float32)

    idx_all = [np.nonzero(~mask[i])[0] for i in range(B)]
    counts = np.array([len(ix) for ix in idx_all])
    NP = int(max(64, 4 * -(-counts.max() // 4)))  # ceil to multiple of 4

    in_maps = []
    for c in range(NCORES):
        bsl = slice(c * BL, (c + 1) * BL)
        enc_c = np.ascontiguousarray(enc_bf[:, bsl, :].transpose(2, 1, 0))  # [E, BL, S]
        enc_p = np.zeros((E, BL, NP), bf16)
        am_p = np.full((BL, NP), NEG, np.float32)
        for b in range(BL):
            ix = idx_all[c * BL + b]
            cnt = len(ix)
            if cnt:
                enc_p[:, b, :cnt] = enc_c[:, b, ix]
                am_p[b, :cnt] = 0.0
        hid_c = hidden[bsl].astype(bf16)                                    # [BL, H]
        hidT_c = np.ascontiguousarray(
            hid_c.T.reshape(KC, 128, BL).transpose(1, 0, 2)
        ).reshape(128, KC * BL)
        in_maps.append({
            "enc_t": enc_p,
            "uwT": uwT_np,
            "wwT": wwT_np,
            "hidT": hidT_c,
            "vt": vt_np,
            "bc": bc_np,
            "amask": am_p.reshape(1, BL * NP),
        })
    return in_maps, NP, idx_all, counts


def _run(in_maps, NP, trace=False):
    from concourse import bass_utils
    if NP not in _CACHE:
        _CACHE[NP] = _build_nc(NP)
    nc = _CACHE[NP]
    return bass_utils.run_bass_kernel_spmd(
        nc, in_maps, core_ids=list(range(NCORES)), trace=trace
    )


def kernel(hidden, encoder_outputs, mask, W_w, W_b, U_w, U_b, v_w,
           _trace=False, _return_bkr=False):
    hidden = np.asarray(hidden, dtype=np.float32)
    encoder_outputs = np.asarray(encoder_outputs, dtype=np.float32)
    mask = np.asarray(mask).astype(bool)
    W_w = np.asarray(W_w, dtype=np.float32)
    W_b = np.asarray(W_b, dtype=np.float32)
    U_w = np.asarray(U_w, dtype=np.float32)
    U_b = np.asarray(U_b, dtype=np.float32)
    v_w = np.asarray(v_w, dtype=np.float32)

    in_maps, NP, idx_all, counts = _prep_inputs(
        hidden, encoder_outputs, mask, W_w, W_b, U_w, U_b, v_w)
    bkr = _run(in_maps, NP, trace=_trace)

    out = np.zeros((B, S), np.float32)
    for c in range(NCORES):
        dev = bkr.results[c]["out"].reshape(BL, NP)
        for b in range(BL):
            i = c * BL + b
            cnt = counts[i]
            if cnt:
                out[i, idx_all[i]] = dev[b, :cnt]
            else:
                # fully-masked row: softmax over all -1e10 is uniform
                out[i, :] = np.float32(1.0 / S)
    if _return_bkr:
        return out, bkr
    return out


# revision 50
# speedup vs baseline: 1.0241x; 1.0177x over previous
"""Bahdanau-attention kernel for Trainium2 (8 NeuronCores, Bass/Tile).

Computation (reference, fp32):
    Wh  = hidden @ W_w.T + W_b                      # [B, H]
    Ue  = einsum('bse,he->bsh', enc^T, U_w) + U_b   # [B, S, H]
    en  = tanh(Wh[:,None,:] + Ue) @ v_w[0]          # [B, S]
    out = softmax(where(mask, -1e10, en), axis=1)

Strategy
- Data-parallel over batch: 8 batches per core, weights replicated.
- Masked positions contribute exactly 0 to the softmax (exp(-1e10) = 0
  in fp32), so the host packs only the unmasked s-columns per batch row
  (padded to NP = max unmasked count rounded to a multiple of 4) and
  scatters results back; the device computes energies only for packed
  columns. This is exact, not an approximation. Fully-masked rows are
  uniform 1/S by definition and fixed up on the host.
- Main matmul out[h, s] = U_w.T-chunk (stationary) x enc-chunk (moving)
  in bf16 with fp32 PSUM accumulation; 16 k-chunks of 128 accumulate in
  one PSUM bank per (batch, h-chunk). Weights are host-rechunked per
  h-chunk so the first main block only needs 0.5 MB of weight DMA.
- Wh + W_b + U_b is folded into the tanh as a per-partition ACT bias;
  the Wh chains interleave with batch 0's main blocks so PE work paces
  the startup DMA stream (which is HBM-bandwidth-bound).
- The v-dots are M=1 matmuls accumulated over h-chunks, batched at the
  end of each batch so the main matmuls stay dense (no per-chunk weight
  switching).
- Per-row softmax runs on partition 0 (no max-subtraction needed:
  |energy| < 32 so fp32 exp is safe; masked/padded columns give 0).

Host-side prep only reshapes/retypes/packs inputs; all FLOPs of the
module run on device in bf16/fp32.
"""

import numpy as np
import ml_dtypes

B, S, H, E = 64, 512, 1024, 2048
NCORES = 8
BL = B // NCORES          # batches per core
HC = H // 128             # h chunks
EC = E // 128             # e (contraction) chunks
KC = H // 128             # k chunks for the Wh matmul
NEG = np.float32(-1e10)

bf16 = ml_dtypes.bfloat16

_CACHE = {}


def _build_nc(NP):
    """Per-core program; NP = packed s-width (padded s-width, multiple of 4, <= 512)."""
    import concourse.mybir as mybir
    import concourse.tile as tile
    from concourse import bacc

    F32 = mybir.dt.float32
    BF = mybir.dt.bfloat16
    AF = mybir.ActivationFunctionType

    nc = bacc.Bacc(num_swdge_queues=4)
    enc_t = nc.declare_dram_parameter("enc_t", [E, BL, NP], BF, isOutput=False)
    # U_w.T pre-chunked by h-chunk: [hc, p(=e%128), ec, v(=h%128)], so the
    # first main block only needs the hc=0 slice (0.5 MB) instead of 4 MB
    uwT = nc.declare_dram_parameter("uwT", [HC, 128, EC, 128], BF, isOutput=False)
    # W_w.T re-chunked the same way: [hc, p(=k%128), kc, v(=h%128)]
    wwT = nc.declare_dram_parameter("wwT", [HC, 128, KC, 128], BF, isOutput=False)
    hidT = nc.declare_dram_parameter("hidT", [128, KC * BL], BF, isOutput=False)
    vt = nc.declare_dram_parameter("vt", [128, HC], BF, isOutput=False)
    bc = nc.declare_dram_parameter("bc", [128, HC], F32, isOutput=False)
    amask = nc.declare_dram_parameter("amask", [1, BL * NP], F32, isOutput=False)
    out_d = nc.declare_dram_parameter("out", [1, BL * NP], F32, isOutput=True)

    enc_r = enc_t.rearrange("(ec p) b s -> ec p b s", p=128)

    ENC_BUFS = 44

    with tile.TileContext(nc) as tc:
        with (
            tc.tile_pool(name="const", bufs=1) as cst,
            tc.tile_pool(name="wpool", bufs=1) as wp,
            tc.tile_pool(name="encp", bufs=ENC_BUFS) as encp,
            tc.tile_pool(name="thp", bufs=4) as thp,
            tc.tile_pool(name="accp", bufs=3) as accp,
            tc.tile_pool(name="smp", bufs=4) as smp,
            tc.tile_pool(name="pup", bufs=5, space="PSUM") as pup,
            tc.tile_pool(name="pep", bufs=2, space="PSUM") as pep,
            tc.tile_pool(name="pwp", bufs=1, space="PSUM") as pwp,
        ):
            # ---- constants / weights -------------------------------------
            # DMA order matters for the startup critical path: the Wh
            # prologue needs hid+ww first; the first main block needs uw.
            hid_sb = cst.tile([128, KC * BL], BF, tag="hid")
            nc.sync.dma_start(hid_sb[:], hidT[:])

            # per-hc weight chunks: each Wh chain / main block only needs its
            # own chunk, so PE work starts after a few hundred KB of DMA and
            # the rest streams in behind it. DMA order matches the b=0
            # interleave: (ww0, uw0, enc0) first, then (ww_k, uw_k) pairs.
            ww_sb = []
            for hc in range(HC):
                t = wp.tile([128, KC * 128], BF, tag=f"ww{hc}")
                ww_sb.append(t)
            uw_sb = []
            for hc in range(HC):
                t = wp.tile([128, EC * 128], BF, tag=f"uw{hc}")
                uw_sb.append(t)
            HALF = EC * 128 // 2
            KHALF = KC * 128 // 2

            nc.sync.dma_start(ww_sb[0][:, 0:KHALF], wwT[0, :, 0:KC // 2, :])
            nc.gpsimd.dma_start(ww_sb[0][:, KHALF:], wwT[0, :, KC // 2:, :])
            nc.sync.dma_start(uw_sb[0][:, 0:HALF], uwT[0, :, 0:EC // 2, :])
            nc.gpsimd.dma_start(uw_sb[0][:, HALF:], uwT[0, :, EC // 2:, :])
            bc_sb = cst.tile([128, HC], F32, tag="bc")
            nc.gpsimd.dma_start(bc_sb[:], bc[:])

            enc0_tiles = []
            for ec in range(EC):
                t = encp.tile([128, NP], BF, tag="enc")
                eng = (nc.sync, nc.gpsimd)[ec % 2]
                eng.dma_start(t[:], enc_r[ec, :, 0, :])
                enc0_tiles.append(t)

            for hc in range(1, HC):
                eng = (nc.sync, nc.gpsimd)[hc % 2]
                eng2 = (nc.gpsimd, nc.sync)[hc % 2]
                eng.dma_start(ww_sb[hc][:], wwT[hc])
                eng.dma_start(uw_sb[hc][:, 0:HALF], uwT[hc, :, 0:EC // 2, :])
                eng2.dma_start(uw_sb[hc][:, HALF:], uwT[hc, :, EC // 2:, :])
            vt_sb = cst.tile([128, HC], BF, tag="vt")
            nc.gpsimd.dma_start(vt_sb[:], vt[:])
            am_sb = cst.tile([1, BL * NP], F32, tag="am")
            nc.gpsimd.dma_start(am_sb[:], amask[:])

            bias_sb = cst.tile([128, HC * BL], F32, tag="bias")
            en_sb = cst.tile([1, BL * NP], F32, tag="en")
            res_sb = cst.tile([1, BL * NP], F32, tag="res")
            ones_sb = cst.tile([128, 1], BF, tag="ones")
            nc.vector.memset(ones_sb[:], 1.0)

            # ---- main loop over local batches ----------------------------
            # b=0 interleaves the Wh/bias prologue chain-by-chain with its
            # own main blocks so PE work paces with the weight DMA stream.
            for b in range(BL):
                if b == 0:
                    enc_tiles = enc0_tiles
                else:
                    enc_tiles = []
                    for ec in range(EC):
                        t = encp.tile([128, NP], BF, tag="enc")
                        eng = nc.sync if ec % 2 == 0 else nc.gpsimd
                        eng.dma_start(t[:], enc_r[ec, :, b, :])
                        enc_tiles.append(t)

                pe_ = pep.tile([1, NP], F32, tag="pe")
                acc = accp.tile([128, NP], F32, tag="acc")
                for hc in range(HC):
                    if b == 0:
                        # Wh chain for this h-chunk, feeding the tanh bias
                        pw = pwp.tile([128, BL], F32, tag="pw")
                        for kc in range(KC):
                            nc.tensor.matmul(
                                pw[:],
                                lhsT=ww_sb[hc][:, kc * 128:(kc + 1) * 128],
                                rhs=hid_sb[:, kc * BL:(kc + 1) * BL],
                                start=(kc == 0),
                                stop=(kc == KC - 1),
                            )
                        nc.vector.tensor_tensor(
                            bias_sb[:, hc * BL:(hc + 1) * BL], pw[:],
                            bc_sb[:, hc:hc + 1].to_broadcast([128, BL]),
                            mybir.AluOpType.add,
                        )
                    pu = pup.tile([128, NP], F32, tag="pu")
                    for ec in range(EC):
                        nc.tensor.matmul(
                            pu[:],
                            lhsT=uw_sb[hc][:, ec * 128:(ec + 1) * 128],
                            rhs=enc_tiles[ec][:],
                            start=(ec == 0),
                            stop=(ec == EC - 1),
                        )
                    th = thp.tile([128, NP], BF, tag="th")
                    nc.scalar.activation(
                        th[:], pu[:], AF.Tanh,
                        bias=bias_sb[:, hc * BL + b:hc * BL + b + 1],
                    )
                    # v-weighting on the (otherwise idle) Vector engine:
                    # acc[p, s] += v[hc*128+p] * tanh[p, s]
                    vcol = vt_sb[:, hc:hc + 1].to_broadcast([128, NP])
                    if hc == 0:
                        nc.vector.tensor_tensor(
                            acc[:], th[:], vcol, mybir.AluOpType.mult)
                    else:
                        tmp = thp.tile([128, NP], F32, tag="tmp")
                        nc.vector.tensor_tensor(
                            tmp[:], th[:], vcol, mybir.AluOpType.mult)
                        nc.vector.tensor_add(acc[:], acc[:], tmp[:])
                # single partition-reduce matmul replaces the 8 v-dots
                accb = thp.tile([128, NP], BF, tag="accb")
                nc.vector.tensor_copy(accb[:], acc[:])
                nc.tensor.matmul(
                    pe_[0:1, :], lhsT=ones_sb[:, 0:1], rhs=accb[:],
                    start=True, stop=True,
                )

                # ---- mask + softmax over packed columns on partition 0 ---
                sl = slice(b * NP, (b + 1) * NP)
                nc.vector.tensor_add(en_sb[0:1, sl], pe_[0:1, :], am_sb[0:1, sl])
                ssum = smp.tile([1, 1], F32, tag="ssum")
                nc.scalar.activation(
                    res_sb[0:1, sl], en_sb[0:1, sl], AF.Exp,
                    accum_out=ssum[0:1, 0:1],
                )
                rcp = smp.tile([1, 1], F32, tag="rcp")
                nc.vector.reciprocal(rcp[0:1, :], ssum[0:1, :])
                nc.vector.tensor_tensor(
                    res_sb[0:1, sl], res_sb[0:1, sl],
                    rcp[0:1, 0:1].to_broadcast([1, NP]),
                    mybir.AluOpType.mult,
                )
                nc.sync.dma_start(out_d[0:1, sl], res_sb[0:1, sl])

    nc.finalize()
    return nc


def _prep_inputs(hidden, encoder_outputs, mask, W_w, W_b, U_w, U_b, v_w):
    enc_bf = encoder_outputs.astype(bf16)          # [S, B, E]
    uwT_np = np.ascontiguousarray(U_w.T).astype(bf16)
    # re-chunk U_w.T [E, H] -> [hc, p, ec, v]: (e=ec*128+p, h=hc*128+v)
    uwT_np = np.ascontiguousarray(
        uwT_np.reshape(EC, 128, HC, 128).transpose(2, 1, 0, 3))
    wwT_np = np.ascontiguousarray(W_w.T).astype(bf16)
    wwT_np = np.ascontiguousarray(
        wwT_np.reshape(KC, 128, HC, 128).transpose(2, 1, 0, 3))
    vt_np = np.ascontiguousarray(v_w[0].reshape(HC, 128).T).astype(bf16)
    bc_np = np.ascontiguousarray((W_b + U_b).reshape(HC, 128).T).astype(np.float32)

    idx_all = [np.nonzero(~mask[i])[0] for i in range(B)]
    counts = np.array([len(ix) for ix in idx_all])
    NP = int(max(64, 4 * -(-counts.max() // 4)))  # ceil to multiple of 4

    in_maps = []
    for c in range(NCORES):
        bsl = slice(c * BL, (c + 1) * BL)
        enc_c = np.ascontiguousarray(enc_bf[:, bsl, :].transpose(2, 1, 0))  # [E, BL, S]
        enc_p = np.zeros((E, BL, NP), bf16)
        am_p = np.full((BL, NP), NEG, np.float32)
        for b in range(BL):
            ix = idx_all[c * BL + b]
            cnt = len(ix)
            if cnt:
                enc_p[:, b, :cnt] = enc_c[:, b, ix]
                am_p[b, :cnt] = 0.0
        hid_c = hidden[bsl].astype(bf16)                                    # [BL, H]
        hidT_c = np.ascontiguousarray(
            hid_c.T.reshape(KC, 128, BL).transpose(1, 0, 2)
        ).reshape(128, KC * BL)
        in_maps.append({
            "enc_t": enc_p,
            "uwT": uwT_np,
            "wwT": wwT_np,
            "hidT": hidT_c,
            "vt": vt_np,
            "bc": bc_np,
            "amask": am_p.reshape(1, BL * NP),
        })
    return in_maps, NP, idx_all, counts


def _run(in_maps, NP, trace=False):
    from concourse import bass_utils
    if NP not in _CACHE:
        _CACHE[NP] = _build_nc(NP)
    nc = _CACHE[NP]
    return bass_utils.run_bass_kernel_spmd(
        nc, in_maps, core_ids=list(range(NCORES)), trace=trace
    )


def kernel(hidden, encoder_outputs, mask, W_w, W_b, U_w, U_b, v_w,
           _trace=False, _return_bkr=False):
    hidden = np.asarray(hidden, dtype=np.float32)
    encoder_outputs = np.asarray(encoder_outputs, dtype=np.float32)
    mask = np.asarray(mask).astype(bool)
    W_w = np.asarray(W_w, dtype=np.float32)
    W_b = np.asarray(W_b, dtype=np.float32)
    U_w = np.asarray(U_w, dtype=np.float32)
    U_b = np.asarray(U_b, dtype=np.float32)
    v_w = np.asarray(v_w, dtype=np.float32)

    in_maps, NP, idx_all, counts = _prep_inputs(
        hidden, encoder_outputs, mask, W_w, W_b, U_w, U_b, v_w)
    bkr = _run(in_maps, NP, trace=_trace)

    out = np.zeros((B, S), np.float32)
    for c in range(NCORES):
        dev = bkr.results[c]["out"].reshape(BL, NP)
        for b in range(BL):
            i = c * BL + b
            cnt = counts[i]
            if cnt:
                out[i, idx_all[i]] = dev[b, :cnt]
            else:
                # fully-masked row: softmax over all -1e10 is uniform
                out[i, :] = np.float32(1.0 / S)
    if _return_bkr:
        return out, bkr
    return out
